# revision 17
# baseline (speedup 1.0000x reference)
"""GAT layer (nn_GATlayer) on 8 Trainium2 NeuronCores via Bass/Tile.

Strategy (edge parallelism over dst-sorted edges, per the sharding hint):
  - Host: sort edges by dst, shard dst nodes contiguously across 8 cores
    (6250 nodes/core), partition each core's nodes into blocks of 128.
    Each block's edges are padded to T tiles of 128 edges (T = global max).
  - Device, per block:
      * T indirect-DMA gathers fetch h[src] 1KB slabs (one edge per
        SBUF partition)
      * selection mask S_T[e, t*128+n] = (dst_local[e,t] == n) via DVE
        compare of host-prepped dst arrays against an iota constant
      * edge logit a = (slab_slot0 . w1) + b[dst]; the dst-side per-node
        dot b[n] = h_block[n,0:D] . w2 is broadcast to edges by
        transposing b on the PE and selecting through S_T; leaky_relu;
        w = exp(a) (no softmax max-shift: logits are O(5) and softmax is
        shift-invariant, fp32 exp handles it exactly)
      * PE matmuls S_w^T @ slab accumulate messages and denominator s in
        PSUM across tiles; out = msg/s; empty nodes (s==0) keep h.
  - Padding edges carry dst_local=SENT so their mask columns are all-zero:
    they contribute exactly nothing.
"""
import math

import numpy as np

import concourse.bacc as bacc
import concourse.bass as bass
import concourse.tile as tile
from concourse import mybir
from concourse.bass import AP
from concourse.bass_utils import run_bass_kernel_spmd

import bass_rust

# problem dims (overridable for small-scale sim tests)
CFG = dict(N=50000, S=2, D=128, n_cores=8)
SLOPE = 0.98
P = 128
SENT = 200.0

_f32 = mybir.dt.float32
_i32 = mybir.dt.int32
_bf16 = mybir.dt.bfloat16


def _dims():
    n, s, d, ncores = CFG["N"], CFG["S"], CFG["D"], CFG["n_cores"]
    return n, s, d, s * d, ncores, n // ncores, math.ceil(n // ncores / P)


def _split_sync_waits(nc, max_waits=1):
    """walrus in this container allows only 1 sync wait per instruction;
    move overflow waits onto preceding nops on the same engine."""
    for f in nc.m.functions:
        for bb in f.blocks:
            insts = bb.instructions
            i = 0
            while i < len(insts):
                ins = insts[i]
                si = ins.sync_info
                if si is not None and len(si.on_wait) > max_waits:
                    waits = list(si.on_wait)
                    si.on_wait = waits[-max_waits:]
                    overflow = waits[:-max_waits]
                    eng = nc.engines[ins.engine]
                    nops = []
                    for j in range(0, len(overflow), max_waits):
                        nop = eng.nop(hint="split_wait", nofuse=True)
                        nop.ins.sync_info = bass_rust.SyncInfo(
                            on_wait=overflow[j : j + max_waits], on_update=[]
                        )
                        nops.append(nop.ins)
                    for f2 in nc.m.functions:
                        for bb2 in f2.blocks:
                            bb2.instructions[:] = [
                                x for x in bb2.instructions if x not in nops
                            ]
                    for k, nop_ins in enumerate(nops):
                        insts.insert(i + k, nop_ins)
                    i += len(nops)
                i += 1


def host_prep(src, dst):
    """Sort edges by dst; build per-core index arrays."""
    n, s, d, sd, ncores, npc, nb = _dims()
    order = np.argsort(dst, kind="stable")
    s_src = np.ascontiguousarray(src[order]).astype(np.int64)
    s_dst = np.ascontiguousarray(dst[order]).astype(np.int64)

    bounds = []
    for c in range(ncores):
        for b in range(nb):
            lo = c * npc + b * P
            hi = min(lo + P, (c + 1) * npc)
            e0 = np.searchsorted(s_dst, lo, side="left")
            e1 = np.searchsorted(s_dst, hi, side="left")
            bounds.append((int(e0), int(e1), lo, hi - lo))
    T = max(1, max(math.ceil((e1 - e0) / P) for e0, e1, _, _ in bounds))

    per_core = []
    for c in range(ncores):
        TT = nb * T
        src_cm = np.zeros((P, TT), np.int32)
        dst_cm = np.full((P, TT), SENT, np.float32)
        for b in range(nb):
            e0, e1, lo, _ = bounds[c * nb + b]
            cnt = e1 - e0
            nt = math.ceil(cnt / P)
            if nt == 0:
                continue
            sbuf = np.zeros(nt * P, np.int64)
            sbuf[:cnt] = s_src[e0:e1]
            src_cm[:, b * T : b * T + nt] = sbuf.reshape(nt, P).T
            dbuf = np.full(nt * P, SENT, np.float32)
            dbuf[:cnt] = (s_dst[e0:e1] - lo).astype(np.float32)
            dst_cm[:, b * T : b * T + nt] = dbuf.reshape(nt, P).T
        per_core.append({"src_cm": src_cm, "dst_cm": dst_cm})
    return T, per_core


def build_program(T, reps=1):
    """Build the SPMD Bass program for tile-count T (v5: bf16 hot path)."""
    import ml_dtypes  # noqa: F401
    n, s, d, sd, ncores, npc, nb = _dims()
    TT = nb * T
    sda = sd + 1  # slab row with appended ones column
    nc = bacc.Bacc("TRN2", target_bir_lowering=False, debug=False,
                   num_devices=ncores)
    h2d = nc.dram_tensor("h", [n, sd + 1], _bf16, kind="ExternalInput").ap()
    wb_d = nc.dram_tensor("wb", [P, 2 * d], _f32, kind="ExternalInput").ap()
    w1b_d = nc.dram_tensor("w1b", [P, d], _bf16, kind="ExternalInput").ap()
    irb_d = nc.dram_tensor("irb", [P, P], _bf16, kind="ExternalInput").ap()
    id_d = nc.dram_tensor("id128", [P, P], _f32, kind="ExternalInput").ap()
    or_d = nc.dram_tensor("ones_row", [1, P], _f32, kind="ExternalInput").ap()
    srccm_d = nc.dram_tensor("src_cm", [P, TT], _i32, kind="ExternalInput").ap()
    dstcm_d = nc.dram_tensor("dst_cm", [P, TT], _bf16, kind="ExternalInput").ap()
    mask_d = nc.dram_tensor("mask_cm", [P, TT * P], _bf16,
                            kind="ExternalInput").ap()
    hcm_d = nc.dram_tensor("hblk_cm", [P, nb * sd], _f32,
                           kind="ExternalInput").ap()
    y_d = nc.dram_tensor("y", [npc, sd], _f32, kind="ExternalOutput").ap()

    with tile.TileContext(nc) as tc:
        with (
            tc.tile_pool(name="const", bufs=1) as cpool,
            tc.tile_pool(name="psum", bufs=2, space="PSUM") as ppool,
        ):
            src_cm = cpool.tile([P, TT], _i32)
            nc.sync.dma_start(out=src_cm[:], in_=srccm_d[:])
            dst_cm = cpool.tile([P, TT], _bf16)
            nc.sync.dma_start(out=dst_cm[:], in_=dstcm_d[:])
            wb = cpool.tile([P, 2 * d], _f32)
            nc.sync.dma_start(out=wb[:], in_=wb_d[:])
            w1b = cpool.tile([P, d], _bf16)
            nc.sync.dma_start(out=w1b[:], in_=w1b_d[:])
            irb = cpool.tile([P, P], _bf16)
            nc.sync.dma_start(out=irb[:], in_=irb_d[:])
            id128 = cpool.tile([P, P], _f32)
            nc.sync.dma_start(out=id128[:], in_=id_d[:])
            ones_row = cpool.tile([1, P], _f32)
            nc.sync.dma_start(out=ones_row[:], in_=or_d[:])
            hcm = cpool.tile([P, nb * sd], _f32)
            nc.sync.dma_start(out=hcm[:], in_=hcm_d[:])
            b_cols = cpool.tile([P, nb], _f32)

            # phase 0: b[n] = h[n, 0:d] . w2 for all this core's nodes
            with tc.tile_pool(name="init", bufs=1) as ipool:
                bmul_all = ipool.tile([P, nb * d], _f32)
                hcm_ap = hcm[:]
                h0_v = AP(hcm_ap.tensor, hcm_ap.offset,
                          [hcm_ap.ap[0], (sd, nb), (1, d)])
                w2_sl = wb[:, d : 2 * d]
                w2_rep = AP(w2_sl.tensor, w2_sl.offset,
                            [w2_sl.ap[0], (0, nb), (1, d)])
                nc.vector.tensor_tensor(
                    out=bmul_all[:], in0=h0_v, in1=w2_rep,
                    op=mybir.AluOpType.mult,
                )
                bm = bmul_all[:]
                bm_v = AP(bm.tensor, bm.offset, [bm.ap[0], (d, nb), (1, d)])
                nc.vector.tensor_reduce(
                    out=b_cols[:], in_=bm_v, axis=mybir.AxisListType.X,
                    op=mybir.AluOpType.add,
                )

            with tc.tile_pool(name="work", bufs=5) as wpool:
                def stage_a(b):
                    """gathers + masks + logits + S_w for block b."""
                    g0 = b * T
                    slab = wpool.tile([P, T, sda], _bf16, tag="slab")
                    for t in range(T):
                        nc.gpsimd.indirect_dma_start(
                            out=slab[:, t, 0:sda],
                            out_offset=None,
                            in_=h2d[:],
                            in_offset=bass.IndirectOffsetOnAxis(
                                ap=src_cm[:, g0 + t : g0 + t + 1], axis=0
                            ),
                        )

                    # S_T[e, t*P+n] = (dst_cm[e, g0+t] == n), host-prepped
                    s_t = wpool.tile([P, T * P], _bf16, tag="s_t")
                    nc.sync.dma_start(
                        out=s_t[:],
                        in_=mask_d[:][:, g0 * P : (g0 + T) * P],
                    )

                    # broadcast this block's b value row to all partitions
                    bt_ps = ppool.tile([1, P], _f32, tag="bt")
                    nc.tensor.matmul(out=bt_ps[:],
                                     lhsT=b_cols[:, b : b + 1],
                                     rhs=id128[:], start=True, stop=True)
                    b_row = wpool.tile([1, P], _f32, tag="b_row")
                    nc.vector.tensor_copy(out=b_row[:], in_=bt_ps[:])
                    bb_ps = ppool.tile([P, P], _f32, tag="bb")
                    nc.tensor.matmul(out=bb_ps[:], lhsT=ones_row[:],
                                     rhs=b_row[:], start=True, stop=True)
                    b_bc = wpool.tile([P, P], _bf16, tag="b_bc")
                    nc.vector.tensor_copy(out=b_bc[:], in_=bb_ps[:])

                    # select b[dst_e]: bsl = S_T * b_bc, b_mat = sum_n
                    bsl = wpool.tile([P, T * P], _bf16, tag="bsl")
                    bb_sl = b_bc[:, 0:P]
                    bb_rep = AP(bb_sl.tensor, bb_sl.offset,
                                [bb_sl.ap[0], (0, T), (1, P)])
                    nc.vector.tensor_tensor(
                        out=bsl[:], in0=s_t[:], in1=bb_rep,
                        op=mybir.AluOpType.mult,
                    )
                    b_mat = wpool.tile([P, T], _f32, tag="b_mat")
                    bsl_ap = bsl[:]
                    bsl_v = AP(bsl_ap.tensor, bsl_ap.offset,
                               [bsl_ap.ap[0], (P, T), (1, P)])
                    nc.vector.tensor_reduce(
                        out=b_mat[:], in_=bsl_v, axis=mybir.AxisListType.X,
                        op=mybir.AluOpType.add,
                    )

                    # src-side dot
                    dmul = wpool.tile([P, T * d], _bf16, tag="dmul")
                    w1_sl = w1b[:, 0:d]
                    w1_rep = AP(w1_sl.tensor, w1_sl.offset,
                                [w1_sl.ap[0], (0, T), (1, d)])
                    sl_ap = slab[:]
                    sl_v = AP(sl_ap.tensor, sl_ap.offset,
                              [sl_ap.ap[0], (sda, T), (1, d)])
                    nc.vector.tensor_tensor(
                        out=dmul[:], in0=sl_v, in1=w1_rep,
                        op=mybir.AluOpType.mult,
                    )
                    dot = wpool.tile([P, T], _f32, tag="dot")
                    dm = dmul[:]
                    dmul_v = AP(dm.tensor, dm.offset,
                                [dm.ap[0], (d, T), (1, d)])
                    nc.vector.tensor_reduce(
                        out=dot[:], in_=dmul_v, axis=mybir.AxisListType.X,
                        op=mybir.AluOpType.add,
                    )

                    # a = dot + b ; leaky (ACT) ; w = exp(a) (ACT, bf16 out)
                    a_mat = wpool.tile([P, T], _f32, tag="a_mat")
                    nc.vector.tensor_tensor(
                        out=a_mat[:], in0=dot[:], in1=b_mat[:],
                        op=mybir.AluOpType.add,
                    )
                    nc.vector.scalar_tensor_tensor(
                        out=a_mat[:], in0=a_mat[:], scalar=SLOPE,
                        in1=a_mat[:], op0=mybir.AluOpType.mult,
                        op1=mybir.AluOpType.max,
                    )
                    w_mat = wpool.tile([P, T], _f32, tag="w_mat")
                    nc.scalar.activation(
                        out=w_mat[:], in_=a_mat[:],
                        func=mybir.ActivationFunctionType.Exp,
                    )
                    return slab, s_t, w_mat

                def stage_bc(b, slab, s_t, w_mat):
                    """PE accumulation + normalization + store for block b.
                    ACT materializes S_w = mask * w per tile so DVE never
                    touches the [P, T*P] mask after the compare."""
                    node_lo = b * P
                    nrows = min(P, npc - node_lo)
                    h_block = hcm[:, b * sd : (b + 1) * sd]
                    acc = ppool.tile([P, sda], _f32, tag="acc", bufs=3)
                    for t in range(T):
                        sw_st = wpool.tile([P, P], _bf16, tag="sw_st",
                                           bufs=8)
                        nc.scalar.activation(
                            out=sw_st[:], in_=s_t[:, t * P : (t + 1) * P],
                            func=mybir.ActivationFunctionType.Copy,
                            scale=w_mat[:, t : t + 1],
                        )
                        nc.tensor.matmul(
                            out=acc[:],
                            lhsT=sw_st[:],
                            rhs=slab[:, t, :],
                            start=(t == 0), stop=(t == T - 1),
                        )
                    s_col = acc[:, sd : sd + 1]

                    eq0 = wpool.tile([P, 1], _f32, tag="eq0")
                    nc.vector.tensor_scalar(
                        out=eq0[:], in0=s_col, scalar1=0.0,
                        scalar2=None, op0=mybir.AluOpType.is_equal,
                    )
                    s_safe = wpool.tile([P, 1], _f32, tag="s_safe")
                    nc.vector.tensor_tensor(
                        out=s_safe[:], in0=s_col, in1=eq0[:],
                        op=mybir.AluOpType.add,
                    )
                    rec = wpool.tile([P, 1], _f32, tag="rec")
                    nc.vector.reciprocal(out=rec[:], in_=s_safe[:])
                    hmask = wpool.tile([P, sd], _f32, tag="hmask")
                    nc.vector.tensor_scalar(
                        out=hmask[:], in0=h_block, scalar1=eq0[:, 0:1],
                        scalar2=None, op0=mybir.AluOpType.mult,
                    )
                    out_sb = wpool.tile([P, sd], _f32, tag="out_sb")
                    nc.vector.scalar_tensor_tensor(
                        out=out_sb[:], in0=acc[:, 0:sd], scalar=rec[:, 0:1],
                        in1=hmask[:], op0=mybir.AluOpType.mult,
                        op1=mybir.AluOpType.add,
                    )
                    nc.sync.dma_start(
                        out=y_d[:][node_lo : node_lo + nrows, :],
                        in_=out_sb[:nrows, :],
                    )

                for rep in range(reps):
                    pending = None
                    for b in range(nb):
                        state = stage_a(b)
                        if pending is not None:
                            stage_bc(pending[0], *pending[1])
                        pending = (b, state)
                    stage_bc(pending[0], *pending[1])

    nc.compile()
    _split_sync_waits(nc, max_waits=1)
    return nc


_cache = {}


def make_in_maps(h_features, w_att, per_core):
    import ml_dtypes
    bf16 = np.dtype(ml_dtypes.bfloat16)
    n, s, d, sd, ncores, npc, nb = _dims()
    h2 = np.ascontiguousarray(h_features.reshape(n, sd), dtype=np.float32)
    h2b = np.ones((n, sd + 1), np.float32)
    h2b[:, 0:sd] = h2
    h2b = h2b.astype(bf16)
    w_flat = np.ascontiguousarray(w_att.reshape(1, 2 * d), dtype=np.float32)
    wb = np.repeat(w_flat, P, axis=0)
    w1b = np.ascontiguousarray(wb[:, 0:d]).astype(bf16)
    irb = np.repeat(np.arange(P, dtype=np.float32).reshape(1, P), P,
                    axis=0).astype(bf16)
    id128 = np.eye(P, dtype=np.float32)
    ones_row = np.ones((1, P), np.float32)
    hcm_list = []
    for c in range(ncores):
        pad_rows = nb * P
        hp = np.zeros((pad_rows, sd), np.float32)
        hp[:npc] = h2[c * npc : (c + 1) * npc]
        hcm_list.append(
            np.ascontiguousarray(
                hp.reshape(nb, P, sd).transpose(1, 0, 2).reshape(P, nb * sd)
            )
        )
    iota_p = np.arange(P, dtype=np.float32)
    mask_list = []
    for c in range(ncores):
        dcm = per_core[c]["dst_cm"]  # [P, TT] float block-local dst
        m = (dcm[:, :, None] == iota_p[None, None, :])
        mask_list.append(
            np.ascontiguousarray(m.reshape(P, -1)).astype(bf16)
        )
    in_maps = []
    for c in range(ncores):
        in_maps.append(
            {
                "h": h2b,
                "mask_cm": mask_list[c],
                "wb": wb,
                "w1b": w1b,
                "irb": irb,
                "id128": id128,
                "ones_row": ones_row,
                "src_cm": per_core[c]["src_cm"],
                "dst_cm": per_core[c]["dst_cm"].astype(bf16),
                "hblk_cm": hcm_list[c],
            }
        )
    return in_maps


def kernel(h_features, src, dst, w_att):
    n, s, d, sd, ncores, npc, nb = _dims()
    h_features = np.ascontiguousarray(h_features, dtype=np.float32)
    src = np.ascontiguousarray(src, dtype=np.int32)
    dst = np.ascontiguousarray(dst, dtype=np.int32)
    w_att = np.ascontiguousarray(w_att, dtype=np.float32)

    T, per_core = host_prep(src, dst)
    if T not in _cache:
        _cache[T] = build_program(T)
    nc = _cache[T]

    in_maps = make_in_maps(h_features, w_att, per_core)
    res = run_bass_kernel_spmd(nc, in_maps, list(range(ncores)))
    out = np.concatenate([res.results[c]["y"] for c in range(ncores)], axis=0)
    return out.reshape(n, s, d).astype(np.float32)



# revision 20
# speedup vs baseline: 1.0602x; 1.0602x over previous
"""GAT layer (nn_GATlayer) on 8 Trainium2 NeuronCores via Bass/Tile.

Strategy (edge parallelism over dst-sorted edges, per the sharding hint):
  - Host: sort edges by dst, shard dst nodes contiguously across 8 cores
    (6250 nodes/core), partition each core's nodes into blocks of 128.
    Each block's edges are padded to T tiles of 128 edges (T = global max).
  - Device, per block:
      * T indirect-DMA gathers fetch h[src] 1KB slabs (one edge per
        SBUF partition)
      * selection mask S_T[e, t*128+n] = (dst_local[e,t] == n) via DVE
        compare of host-prepped dst arrays against an iota constant
      * edge logit a = (slab_slot0 . w1) + b[dst]; the dst-side per-node
        dot b[n] = h_block[n,0:D] . w2 is broadcast to edges by
        transposing b on the PE and selecting through S_T; leaky_relu;
        w = exp(a) (no softmax max-shift: logits are O(5) and softmax is
        shift-invariant, fp32 exp handles it exactly)
      * PE matmuls S_w^T @ slab accumulate messages and denominator s in
        PSUM across tiles; out = msg/s; empty nodes (s==0) keep h.
  - Padding edges carry dst_local=SENT so their mask columns are all-zero:
    they contribute exactly nothing.
"""
import math

import numpy as np

import concourse.bacc as bacc
import concourse.bass as bass
import concourse.tile as tile
from concourse import mybir
from concourse.bass import AP
from concourse.bass_utils import run_bass_kernel_spmd

import bass_rust

# problem dims (overridable for small-scale sim tests)
CFG = dict(N=50000, S=2, D=128, n_cores=8)
SLOPE = 0.98
P = 128
SENT = 200.0

_f32 = mybir.dt.float32
_i32 = mybir.dt.int32
_bf16 = mybir.dt.bfloat16


def _dims():
    n, s, d, ncores = CFG["N"], CFG["S"], CFG["D"], CFG["n_cores"]
    return n, s, d, s * d, ncores, n // ncores, math.ceil(n // ncores / P)


def _split_sync_waits(nc, max_waits=1):
    """walrus in this container allows only 1 sync wait per instruction;
    move overflow waits onto preceding nops on the same engine."""
    for f in nc.m.functions:
        for bb in f.blocks:
            insts = bb.instructions
            i = 0
            while i < len(insts):
                ins = insts[i]
                si = ins.sync_info
                if si is not None and len(si.on_wait) > max_waits:
                    waits = list(si.on_wait)
                    si.on_wait = waits[-max_waits:]
                    overflow = waits[:-max_waits]
                    eng = nc.engines[ins.engine]
                    nops = []
                    for j in range(0, len(overflow), max_waits):
                        nop = eng.nop(hint="split_wait", nofuse=True)
                        nop.ins.sync_info = bass_rust.SyncInfo(
                            on_wait=overflow[j : j + max_waits], on_update=[]
                        )
                        nops.append(nop.ins)
                    for f2 in nc.m.functions:
                        for bb2 in f2.blocks:
                            bb2.instructions[:] = [
                                x for x in bb2.instructions if x not in nops
                            ]
                    for k, nop_ins in enumerate(nops):
                        insts.insert(i + k, nop_ins)
                    i += len(nops)
                i += 1


def host_prep(src, dst):
    """Sort edges by dst; build per-core index arrays."""
    n, s, d, sd, ncores, npc, nb = _dims()
    order = np.argsort(dst, kind="stable")
    s_src = np.ascontiguousarray(src[order]).astype(np.int64)
    s_dst = np.ascontiguousarray(dst[order]).astype(np.int64)

    bounds = []
    for c in range(ncores):
        for b in range(nb):
            lo = c * npc + b * P
            hi = min(lo + P, (c + 1) * npc)
            e0 = np.searchsorted(s_dst, lo, side="left")
            e1 = np.searchsorted(s_dst, hi, side="left")
            bounds.append((int(e0), int(e1), lo, hi - lo))
    T = max(1, max(math.ceil((e1 - e0) / P) for e0, e1, _, _ in bounds))

    per_core = []
    for c in range(ncores):
        TT = nb * T
        src_cm = np.zeros((P, TT), np.int32)
        dst_cm = np.full((P, TT), SENT, np.float32)
        for b in range(nb):
            e0, e1, lo, _ = bounds[c * nb + b]
            cnt = e1 - e0
            nt = math.ceil(cnt / P)
            if nt == 0:
                continue
            sbuf = np.zeros(nt * P, np.int64)
            sbuf[:cnt] = s_src[e0:e1]
            src_cm[:, b * T : b * T + nt] = sbuf.reshape(nt, P).T
            dbuf = np.full(nt * P, SENT, np.float32)
            dbuf[:cnt] = (s_dst[e0:e1] - lo).astype(np.float32)
            dst_cm[:, b * T : b * T + nt] = dbuf.reshape(nt, P).T
        per_core.append({"src_cm": src_cm, "dst_cm": dst_cm})
    return T, per_core


def build_program(T, reps=1):
    """Build the SPMD Bass program for tile-count T (v5: bf16 hot path)."""
    import ml_dtypes  # noqa: F401
    n, s, d, sd, ncores, npc, nb = _dims()
    TT = nb * T
    sda = sd + 1  # slab row with appended ones column
    nc = bacc.Bacc("TRN2", target_bir_lowering=False, debug=False,
                   num_devices=ncores)
    h2d = nc.dram_tensor("h", [n, sd + 1], _bf16, kind="ExternalInput").ap()
    wb_d = nc.dram_tensor("wb", [P, 2 * d], _f32, kind="ExternalInput").ap()
    w1b_d = nc.dram_tensor("w1b", [P, d], _bf16, kind="ExternalInput").ap()
    irb_d = nc.dram_tensor("irb", [P, P], _bf16, kind="ExternalInput").ap()
    id_d = nc.dram_tensor("id128", [P, P], _f32, kind="ExternalInput").ap()
    or_d = nc.dram_tensor("ones_row", [1, P], _f32, kind="ExternalInput").ap()
    srccm_d = nc.dram_tensor("src_cm", [P, TT], _i32, kind="ExternalInput").ap()
    dstcm_d = nc.dram_tensor("dst_cm", [P, TT], _bf16, kind="ExternalInput").ap()
    hcm_d = nc.dram_tensor("hblk_cm", [P, nb * sd], _f32,
                           kind="ExternalInput").ap()
    y_d = nc.dram_tensor("y", [npc, sd], _f32, kind="ExternalOutput").ap()

    with tile.TileContext(nc) as tc:
        with (
            tc.tile_pool(name="const", bufs=1) as cpool,
            tc.tile_pool(name="psum", bufs=2, space="PSUM") as ppool,
        ):
            src_cm = cpool.tile([P, TT], _i32)
            nc.sync.dma_start(out=src_cm[:], in_=srccm_d[:])
            dst_cm = cpool.tile([P, TT], _bf16)
            nc.sync.dma_start(out=dst_cm[:], in_=dstcm_d[:])
            wb = cpool.tile([P, 2 * d], _f32)
            nc.sync.dma_start(out=wb[:], in_=wb_d[:])
            w1b = cpool.tile([P, d], _bf16)
            nc.sync.dma_start(out=w1b[:], in_=w1b_d[:])
            irb = cpool.tile([P, P], _bf16)
            nc.sync.dma_start(out=irb[:], in_=irb_d[:])
            id128 = cpool.tile([P, P], _f32)
            nc.sync.dma_start(out=id128[:], in_=id_d[:])
            ones_row = cpool.tile([1, P], _f32)
            nc.sync.dma_start(out=ones_row[:], in_=or_d[:])
            hcm = cpool.tile([P, nb * sd], _f32)
            nc.sync.dma_start(out=hcm[:], in_=hcm_d[:])
            b_cols = cpool.tile([P, nb], _f32)

            # phase 0: b[n] = h[n, 0:d] . w2 for all this core's nodes
            with tc.tile_pool(name="init", bufs=1) as ipool:
                bmul_all = ipool.tile([P, nb * d], _f32)
                hcm_ap = hcm[:]
                h0_v = AP(hcm_ap.tensor, hcm_ap.offset,
                          [hcm_ap.ap[0], (sd, nb), (1, d)])
                w2_sl = wb[:, d : 2 * d]
                w2_rep = AP(w2_sl.tensor, w2_sl.offset,
                            [w2_sl.ap[0], (0, nb), (1, d)])
                nc.vector.tensor_tensor(
                    out=bmul_all[:], in0=h0_v, in1=w2_rep,
                    op=mybir.AluOpType.mult,
                )
                bm = bmul_all[:]
                bm_v = AP(bm.tensor, bm.offset, [bm.ap[0], (d, nb), (1, d)])
                nc.vector.tensor_reduce(
                    out=b_cols[:], in_=bm_v, axis=mybir.AxisListType.X,
                    op=mybir.AluOpType.add,
                )

            # hoist the per-block b broadcast out of the steady-state loop:
            # b_bc_all[:, b*P:(b+1)*P] = b value row of block b, replicated
            # to all partitions (removes 2 PE matmuls + 2 copies and their
            # PSUM round-trips from every block's critical chain)
            b_bc_all = cpool.tile([P, nb * P], _bf16)
            with tc.tile_pool(name="binit", bufs=3) as bpool, \
                 tc.tile_pool(name="binitp", bufs=2, space="PSUM") as bppool:
                for b in range(nb):
                    bt_ps = bppool.tile([1, P], _f32, tag="bt")
                    nc.tensor.matmul(out=bt_ps[:],
                                     lhsT=b_cols[:, b : b + 1],
                                     rhs=id128[:], start=True, stop=True)
                    b_row = bpool.tile([1, P], _f32, tag="b_row")
                    nc.vector.tensor_copy(out=b_row[:], in_=bt_ps[:])
                    bb_ps = bppool.tile([P, P], _f32, tag="bb")
                    nc.tensor.matmul(out=bb_ps[:], lhsT=ones_row[:],
                                     rhs=b_row[:], start=True, stop=True)
                    nc.vector.tensor_copy(
                        out=b_bc_all[:, b * P : (b + 1) * P], in_=bb_ps[:])

            with tc.tile_pool(name="work", bufs=3) as wpool:
                def stage_a(b):
                    """gathers + masks + logits + S_w for block b."""
                    g0 = b * T
                    slab = wpool.tile([P, T, sda], _bf16, tag="slab")
                    for t in range(T):
                        nc.gpsimd.indirect_dma_start(
                            out=slab[:, t, 0:sda],
                            out_offset=None,
                            in_=h2d[:],
                            in_offset=bass.IndirectOffsetOnAxis(
                                ap=src_cm[:, g0 + t : g0 + t + 1], axis=0
                            ),
                        )

                    # S_T[e, t*P+n] = (dst_cm[e, g0+t] == n)
                    s_t = wpool.tile([P, T * P], _bf16, tag="s_t")
                    dcm_sl = dst_cm[:, g0 : g0 + T]
                    dcm_rep = AP(dcm_sl.tensor, dcm_sl.offset,
                                 dcm_sl.ap + [(0, P)])
                    ir_sl = irb[:, 0:P]
                    ir_rep = AP(ir_sl.tensor, ir_sl.offset,
                                [ir_sl.ap[0], (0, T), (1, P)])
                    nc.vector.tensor_tensor(
                        out=s_t[:], in0=dcm_rep, in1=ir_rep,
                        op=mybir.AluOpType.is_equal,
                    )

                    # select b[dst_e]: bsl = S_T * b_bc, b_mat = sum_n
                    bsl = wpool.tile([P, T * P], _bf16, tag="bsl")
                    bb_sl = b_bc_all[:, b * P : (b + 1) * P]
                    bb_rep = AP(bb_sl.tensor, bb_sl.offset,
                                [bb_sl.ap[0], (0, T), (1, P)])
                    nc.vector.tensor_tensor(
                        out=bsl[:], in0=s_t[:], in1=bb_rep,
                        op=mybir.AluOpType.mult,
                    )
                    b_mat = wpool.tile([P, T], _f32, tag="b_mat")
                    bsl_ap = bsl[:]
                    bsl_v = AP(bsl_ap.tensor, bsl_ap.offset,
                               [bsl_ap.ap[0], (P, T), (1, P)])
                    nc.vector.tensor_reduce(
                        out=b_mat[:], in_=bsl_v, axis=mybir.AxisListType.X,
                        op=mybir.AluOpType.add,
                    )

                    # src-side dot
                    dmul = wpool.tile([P, T * d], _bf16, tag="dmul")
                    w1_sl = w1b[:, 0:d]
                    w1_rep = AP(w1_sl.tensor, w1_sl.offset,
                                [w1_sl.ap[0], (0, T), (1, d)])
                    sl_ap = slab[:]
                    sl_v = AP(sl_ap.tensor, sl_ap.offset,
                              [sl_ap.ap[0], (sda, T), (1, d)])
                    nc.vector.tensor_tensor(
                        out=dmul[:], in0=sl_v, in1=w1_rep,
                        op=mybir.AluOpType.mult,
                    )
                    dot = wpool.tile([P, T], _f32, tag="dot")
                    dm = dmul[:]
                    dmul_v = AP(dm.tensor, dm.offset,
                                [dm.ap[0], (d, T), (1, d)])
                    nc.vector.tensor_reduce(
                        out=dot[:], in_=dmul_v, axis=mybir.AxisListType.X,
                        op=mybir.AluOpType.add,
                    )

                    # a = dot + b ; leaky (ACT) ; w = exp(a) (ACT, bf16 out)
                    a_mat = wpool.tile([P, T], _f32, tag="a_mat")
                    nc.vector.tensor_tensor(
                        out=a_mat[:], in0=dot[:], in1=b_mat[:],
                        op=mybir.AluOpType.add,
                    )
                    nc.vector.scalar_tensor_tensor(
                        out=a_mat[:], in0=a_mat[:], scalar=SLOPE,
                        in1=a_mat[:], op0=mybir.AluOpType.mult,
                        op1=mybir.AluOpType.max,
                    )
                    w_mat = wpool.tile([P, T], _f32, tag="w_mat")
                    nc.scalar.activation(
                        out=w_mat[:], in_=a_mat[:],
                        func=mybir.ActivationFunctionType.Exp,
                    )
                    return slab, s_t, w_mat

                def stage_bc(b, slab, s_t, w_mat):
                    """PE accumulation + normalization + store for block b.
                    ACT materializes S_w = mask * w per tile so DVE never
                    touches the [P, T*P] mask after the compare."""
                    node_lo = b * P
                    nrows = min(P, npc - node_lo)
                    h_block = hcm[:, b * sd : (b + 1) * sd]
                    acc = ppool.tile([P, sda], _f32, tag="acc", bufs=3)
                    for t in range(T):
                        sw_st = wpool.tile([P, P], _bf16, tag="sw_st",
                                           bufs=4)
                        nc.scalar.activation(
                            out=sw_st[:], in_=s_t[:, t * P : (t + 1) * P],
                            func=mybir.ActivationFunctionType.Copy,
                            scale=w_mat[:, t : t + 1],
                        )
                        nc.tensor.matmul(
                            out=acc[:],
                            lhsT=sw_st[:],
                            rhs=slab[:, t, :],
                            start=(t == 0), stop=(t == T - 1),
                        )
                    s_col = acc[:, sd : sd + 1]

                    eq0 = wpool.tile([P, 1], _f32, tag="eq0")
                    nc.vector.tensor_scalar(
                        out=eq0[:], in0=s_col, scalar1=0.0,
                        scalar2=None, op0=mybir.AluOpType.is_equal,
                    )
                    s_safe = wpool.tile([P, 1], _f32, tag="s_safe")
                    nc.vector.tensor_tensor(
                        out=s_safe[:], in0=s_col, in1=eq0[:],
                        op=mybir.AluOpType.add,
                    )
                    rec = wpool.tile([P, 1], _f32, tag="rec")
                    nc.vector.reciprocal(out=rec[:], in_=s_safe[:])
                    hmask = wpool.tile([P, sd], _f32, tag="hmask")
                    nc.vector.tensor_scalar(
                        out=hmask[:], in0=h_block, scalar1=eq0[:, 0:1],
                        scalar2=None, op0=mybir.AluOpType.mult,
                    )
                    out_sb = wpool.tile([P, sd], _f32, tag="out_sb")
                    nc.vector.scalar_tensor_tensor(
                        out=out_sb[:], in0=acc[:, 0:sd], scalar=rec[:, 0:1],
                        in1=hmask[:], op0=mybir.AluOpType.mult,
                        op1=mybir.AluOpType.add,
                    )
                    nc.sync.dma_start(
                        out=y_d[:][node_lo : node_lo + nrows, :],
                        in_=out_sb[:nrows, :],
                    )

                for rep in range(reps):
                    pending = None
                    for b in range(nb):
                        state = stage_a(b)
                        if pending is not None:
                            stage_bc(pending[0], *pending[1])
                        pending = (b, state)
                    stage_bc(pending[0], *pending[1])

    nc.compile()
    _split_sync_waits(nc, max_waits=1)
    return nc


_cache = {}


def make_in_maps(h_features, w_att, per_core):
    import ml_dtypes
    bf16 = np.dtype(ml_dtypes.bfloat16)
    n, s, d, sd, ncores, npc, nb = _dims()
    h2 = np.ascontiguousarray(h_features.reshape(n, sd), dtype=np.float32)
    h2b = np.ones((n, sd + 1), np.float32)
    h2b[:, 0:sd] = h2
    h2b = h2b.astype(bf16)
    w_flat = np.ascontiguousarray(w_att.reshape(1, 2 * d), dtype=np.float32)
    wb = np.repeat(w_flat, P, axis=0)
    w1b = np.ascontiguousarray(wb[:, 0:d]).astype(bf16)
    irb = np.repeat(np.arange(P, dtype=np.float32).reshape(1, P), P,
                    axis=0).astype(bf16)
    id128 = np.eye(P, dtype=np.float32)
    ones_row = np.ones((1, P), np.float32)
    hcm_list = []
    for c in range(ncores):
        pad_rows = nb * P
        hp = np.zeros((pad_rows, sd), np.float32)
        hp[:npc] = h2[c * npc : (c + 1) * npc]
        hcm_list.append(
            np.ascontiguousarray(
                hp.reshape(nb, P, sd).transpose(1, 0, 2).reshape(P, nb * sd)
            )
        )
    in_maps = []
    for c in range(ncores):
        in_maps.append(
            {
                "h": h2b,
                "wb": wb,
                "w1b": w1b,
                "irb": irb,
                "id128": id128,
                "ones_row": ones_row,
                "src_cm": per_core[c]["src_cm"],
                "dst_cm": per_core[c]["dst_cm"].astype(bf16),
                "hblk_cm": hcm_list[c],
            }
        )
    return in_maps


def kernel(h_features, src, dst, w_att):
    n, s, d, sd, ncores, npc, nb = _dims()
    h_features = np.ascontiguousarray(h_features, dtype=np.float32)
    src = np.ascontiguousarray(src, dtype=np.int32)
    dst = np.ascontiguousarray(dst, dtype=np.int32)
    w_att = np.ascontiguousarray(w_att, dtype=np.float32)

    T, per_core = host_prep(src, dst)
    if T not in _cache:
        _cache[T] = build_program(T)
    nc = _cache[T]

    in_maps = make_in_maps(h_features, w_att, per_core)
    res = run_bass_kernel_spmd(nc, in_maps, list(range(ncores)))
    out = np.concatenate([res.results[c]["y"] for c in range(ncores)], axis=0)
    return out.reshape(n, s, d).astype(np.float32)

